# revision 46
# baseline (speedup 1.0000x reference)
"""Distributed Bass kernel for nn_Attention_57612691309274 on 8 TRN2 NeuronCores.

Reference computes, for x [B=2, S=2048, D=1024], H=16 heads, Dh=64:
  q/k/v = einsum('bsd,hde->bshe', x, W) + b, scaled by 1/sqrt(D)
  scores = q@k^T / sqrt(Dh), causal mask, softmax
  out = ((softmax @ v) @ W_O) * 1/sqrt(Dh) + b_O

Sharding: core c => batch b = c//4, head-group hg = c%4 (heads 4hg..4hg+3).
Each core projects q/k/v for its 4 heads over its batch, runs causal
attention with the (unnormalized z ++ softmax-denominator) trick: the AV
matmul's stationary operand is [v | ones], so row 64 of the PSUM output is
the softmax denominator. Scores are computed transposed ([k, q] layout) so
softmax needs no partition reduction and no max subtraction (scores are
~N(0,1) by umup scaling). Head pairs (even head on PE rows 0-63, odd head
on 64-127) issue score matmuls to disjoint row-groups that run
concurrently, and share one big exp() call.

Each core then applies its own 256-row slice of W_O locally (partial
out-projection) and the per-query-block partial sums are combined with
four chunked ReduceScatter(add) collectives over each group of 4 cores —
issued as each query block completes, so all but the last overlap the
remaining attention compute. Core at group position g receives query rows
[512j + 128g, +128) for each block j; the host reassembles.

All umup scale factors are folded on the host into W_Q (1/8192) and W_O
(1/256). x is fed pre-transposed and pre-cast to bf16.
"""

import os
import sys

if "/opt/trn_rl_repo" not in sys.path:
    sys.path.insert(0, "/opt/trn_rl_repo")

import numpy as np
import ml_dtypes

import concourse.bass as bass
import concourse.tile as tile
from concourse import bacc, mybir

BF16 = np.dtype(ml_dtypes.bfloat16)
F32 = np.float32

B, S, D, H, DH = 2, 2048, 1024, 16, 64
HC = 4            # heads per core
E = HC * DH       # 256 head-dim columns per core
N_CORES = 8
CORE_IDS = list(range(N_CORES))
QB = 512          # query block
P = 128

_NC_CACHE = {}


def build_kernel():
    nc = bacc.Bacc("TRN2", target_bir_lowering=False, debug=False,
                   num_devices=N_CORES)
    dt = mybir.dt

    # ---- external I/O (per-core shards fed from host) ----
    xT_d = nc.dram_tensor("xT", [D, S], dt.bfloat16, kind="ExternalInput")
    wq_d = nc.dram_tensor("wq", [D, E], dt.bfloat16, kind="ExternalInput")
    wk_d = nc.dram_tensor("wk", [D, E], dt.bfloat16, kind="ExternalInput")
    wv_d = nc.dram_tensor("wv", [D, E], dt.bfloat16, kind="ExternalInput")
    wo_d = nc.dram_tensor("wo", [E, D], dt.bfloat16, kind="ExternalInput")
    bq_d = nc.dram_tensor("bq", [P, 2], dt.float32, kind="ExternalInput")
    bk_d = nc.dram_tensor("bk", [P, 2], dt.float32, kind="ExternalInput")
    bv_d = nc.dram_tensor("bv", [P, E], dt.float32, kind="ExternalInput")
    bo_d = nc.dram_tensor("bo", [P, D], dt.float32, kind="ExternalInput")
    tri_d = nc.dram_tensor("tri", [P, P], dt.bfloat16, kind="ExternalInput")
    out_d = nc.dram_tensor("out", [QB, D], dt.bfloat16, kind="ExternalOutput")

    # ---- internal DRAM ----
    rs_in = nc.dram_tensor("rs_in", [S, D], dt.bfloat16)
    rs_out = nc.dram_tensor("rs_out", [QB, D], dt.bfloat16)
    # denominator round-trip buffers (one slot per head-pair x q-block)
    dnd_a = nc.dram_tensor("dnd_a", [8, 2 * QB], dt.float32)
    dnd_b = nc.dram_tensor("dnd_b", [8, 2 * QB], dt.float32)

    groups = [[0, 1, 2, 3], [4, 5, 6, 7]]

    with tile.TileContext(nc) as tc:
        with (
            tc.tile_pool(name="persist", bufs=1) as pp,
            tc.tile_pool(name="etile", bufs=6) as ep,
            tc.tile_pool(name="obuf", bufs=2) as op_,
            tc.tile_pool(name="rb", bufs=2) as rp,
            tc.tile_pool(name="sc", bufs=3, space="PSUM") as scp,
            tc.tile_pool(name="zp", bufs=1, space="PSUM") as zpp,
        ):
            # ---------- resident SBUF loads (weights first: the first
            # projection matmuls need wq + one xT chunk only) ----------
            wq = pp.tile([P, 8, E], dt.bfloat16, tag="wq")
            nc.sync.dma_start(wq[:], wq_d.ap().rearrange("(o p) f -> p o f", p=P))
            wk = pp.tile([P, 8, E], dt.bfloat16, tag="wk")
            nc.sync.dma_start(wk[:], wk_d.ap().rearrange("(o p) f -> p o f", p=P))
            wv = pp.tile([P, 8, E], dt.bfloat16, tag="wv")
            nc.sync.dma_start(wv[:], wv_d.ap().rearrange("(o p) f -> p o f", p=P))
            xT = pp.tile([P, 8, S], dt.bfloat16, tag="xT")
            xT_v = xT_d.ap().rearrange("(o p) f -> p o f", p=P)
            for t in range(8):
                nc.sync.dma_start(xT[:, t], xT_v[:, t])
            wo = pp.tile([P, 2, D], dt.bfloat16, tag="wo")
            nc.sync.dma_start(wo[:], wo_d.ap().rearrange("(o p) f -> p o f", p=P))
            bq = pp.tile([P, 2], dt.float32, tag="bq")
            nc.sync.dma_start(bq[:], bq_d.ap()[:])
            bk = pp.tile([P, 2], dt.float32, tag="bk")
            nc.sync.dma_start(bk[:], bk_d.ap()[:])
            bv = pp.tile([P, E], dt.float32, tag="bv")
            nc.sync.dma_start(bv[:], bv_d.ap()[:])
            bo = pp.tile([P, D], dt.float32, tag="bo")
            nc.sync.dma_start(bo[:], bo_d.ap()[:])
            tri = pp.tile([P, P], dt.bfloat16, tag="tri")
            nc.sync.dma_start(tri[:], tri_d.ap()[:])

            qT = pp.tile([P, 2, S], dt.bfloat16, tag="qT")
            kT = pp.tile([P, 2, S], dt.bfloat16, tag="kT")
            vsb = pp.tile([P, 16, HC * (DH + 1)], dt.bfloat16, tag="vsb")
            # z^T staging, laid out [p, e-tile(2), q-block(4), 512]
            zT = pp.tile([P, 2, 4, QB], dt.bfloat16, tag="zT")

            def qkv_block(jb):
                """Project q, k (e x 512 rows) and v (512 rows x e) for one
                512-row block of the sequence."""
                for w_sb, b_sb, dst in ((wq, bq, qT), (wk, bk, kT)):
                    ps = scp.tile([P, 2 * QB], dt.float32, tag="sc")
                    for m in range(2):
                        for t in range(8):
                            nc.tensor.matmul(
                                ps[:, QB * m:QB * (m + 1)],
                                lhsT=w_sb[:, t, P * m:P * (m + 1)],
                                rhs=xT[:, t, QB * jb:QB * (jb + 1)],
                                start=(t == 0), stop=(t == 7),
                            )
                    nc.vector.tensor_tensor(
                        out=dst[:, :, QB * jb:QB * (jb + 1)],
                        in0=ps.rearrange("p (m f) -> p m f", m=2),
                        in1=b_sb[:, :, None].to_broadcast([P, 2, QB]),
                        op=mybir.AluOpType.add,
                    )
                for half in range(2):       # two v row-tiles per sc tile
                    ps = scp.tile([P, 2 * QB], dt.float32, tag="sc")
                    for sub in range(2):
                        jt = 4 * jb + 2 * half + sub
                        psv = ps[:, QB * sub:QB * sub + E]
                        for t in range(8):
                            nc.tensor.matmul(
                                psv,
                                lhsT=xT[:, t, P * jt:P * (jt + 1)],
                                rhs=wv[:, t, :],
                                start=(t == 0), stop=(t == 7),
                            )
                        nc.vector.memset(vsb[:, jt, :], 1.0)
                        nc.vector.tensor_tensor(
                            out=vsb[:, jt].rearrange(
                                "p (h e) -> p h e", h=HC)[:, :, 0:DH],
                            in0=psv.rearrange("p (h e) -> p h e", h=HC),
                            in1=bv.rearrange("p (h e) -> p h e", h=HC),
                            op=mybir.AluOpType.add,
                        )

            def attention_pair(j, hp):
                """Score/softmax/AV for head pair hp on query block j."""
                q0 = QB * j
                n_kt = 4 * (j + 1)
                pz = zpp.tile([DH + 1, 2 * QB], dt.float32, tag="z")
                for t in range(n_kt):
                    psc = scp.tile([P, 2 * QB], dt.float32, tag="sc")
                    for g in range(2):      # even/odd head -> PE row groups
                        b0 = 64 * g
                        nc.tensor.matmul(
                            psc[:, QB * g:QB * (g + 1)],
                            lhsT=kT[b0:b0 + 64, hp, P * t:P * (t + 1)],
                            rhs=qT[b0:b0 + 64, hp, q0:q0 + QB],
                            start=True, stop=True,
                            tile_position=(b0, 0),
                        )
                    et = ep.tile([P, 2 * QB], dt.bfloat16, tag="et")
                    et3 = et.rearrange("p (g f) -> p g f", g=2)
                    ps3 = psc.rearrange("p (g f) -> p g f", g=2)
                    s = t - 4 * j
                    if s < 0:      # fully unmasked tile
                        nc.scalar.activation(
                            et[:], psc[:], mybir.ActivationFunctionType.Exp)
                    else:          # diagonal-crossing tile
                        if s > 0:
                            nc.vector.memset(et3[:, :, 0:P * s], 0.0)
                        nc.scalar.activation(
                            et3[:, :, P * s:QB], ps3[:, :, P * s:QB],
                            mybir.ActivationFunctionType.Exp)
                        nc.vector.tensor_tensor(
                            out=et3[:, :, P * s:P * (s + 1)],
                            in0=et3[:, :, P * s:P * (s + 1)],
                            in1=tri[:, None, :].to_broadcast([P, 2, P]),
                            op=mybir.AluOpType.mult,
                        )
                    for g in range(2):
                        h = 2 * hp + g
                        nc.tensor.matmul(
                            pz[:, QB * g:QB * (g + 1)],
                            lhsT=vsb[:, t, (DH + 1) * h:(DH + 1) * (h + 1)],
                            rhs=et[:, QB * g:QB * (g + 1)],
                            start=(t == 0), stop=(t == n_kt - 1),
                            skip_group_check=True,
                        )
                # Copy the whole z+denominator PSUM tile to SBUF right away so
                # the PSUM bank recycles fast; normalize from SBUF. The
                # denominators (row 64) round-trip through DRAM to (a)
                # reshape [1, 1024] -> [128, 8] so reciprocal is cheap, and
                # (b) partition-broadcast the result back to 64 rows.
                slot = 2 * j + hp
                pzc = rp.tile([DH + 1, 2 * QB], dt.float32, tag="pzc")
                nc.vector.tensor_copy(pzc[:], pz[:])
                nc.sync.dma_start(
                    dnd_a.ap()[slot:slot + 1, :], pzc[DH:DH + 1, :])
                dnr = rp.tile([P, 8], dt.float32, tag="dnr")
                nc.sync.dma_start(
                    dnr[:], dnd_a.ap()[slot].rearrange("(p f) -> p f", p=P))
                nc.vector.reciprocal(dnr[:], dnr[:])
                nc.sync.dma_start(
                    dnd_b.ap()[slot].rearrange("(p f) -> p f", p=P), dnr[:])
                rb = rp.tile([DH, 2 * QB], dt.float32, tag="rb")
                nc.sync.dma_start(
                    rb[:],
                    dnd_b.ap()[slot:slot + 1, :].to_broadcast([DH, 2 * QB]))
                for g in range(2):
                    nc.vector.tensor_tensor(
                        out=zT[64 * g:64 * (g + 1), hp, j, :],
                        in0=pzc[0:DH, QB * g:QB * (g + 1)],
                        in1=rb[:, QB * g:QB * (g + 1)],
                        op=mybir.AluOpType.mult,
                    )

            def outproj_block(j):
                """Local partial out-projection of query block j (K = my 256
                head-dims), then a chunked ReduceScatter over my group."""
                ob = op_.tile([P, 4, D], dt.bfloat16, tag="opb")
                for qt in range(4):
                    ps = scp.tile([P, 2 * QB], dt.float32, tag="sc")
                    for nb in range(2):
                        for t in range(2):
                            nc.tensor.matmul(
                                ps[:, QB * nb:QB * (nb + 1)],
                                lhsT=zT[:, t, j, P * qt:P * (qt + 1)],
                                rhs=wo[:, t, QB * nb:QB * (nb + 1)],
                                start=(t == 0), stop=(t == 1),
                            )
                    # bo holds b_O/4: each group member adds a quarter, so
                    # the ReduceScatter sum carries the full bias.
                    nc.vector.tensor_tensor(
                        out=ob[:, qt, :], in0=ps[:], in1=bo[:],
                        op=mybir.AluOpType.add)
                nc.sync.dma_start(
                    rs_in.ap()[QB * j:QB * (j + 1), :].rearrange(
                        "(o p) f -> p o f", p=P),
                    ob[:])
                nc.gpsimd.collective_compute(
                    "ReduceScatter",
                    mybir.AluOpType.add,
                    replica_groups=groups,
                    ins=[rs_in.ap()[QB * j:QB * (j + 1), :].opt()],
                    outs=[rs_out.ap()[P * j:P * (j + 1), :].opt()],
                )

            # ---------- main schedule ----------
            qkv_block(0)
            for j in range(4):
                if j < 3:
                    qkv_block(j + 1)
                attention_pair(j, 0)
                if j >= 1:
                    outproj_block(j - 1)
                attention_pair(j, 1)
            outproj_block(3)

            # ---------- final epilogue: bias + f32 cast ----------
            # Runs entirely on gpsimd: these ops wait on the ReduceScatters,
            # and on any other engine queue that wait would head-of-line
            # block compute still in flight.
            # Bias was folded pre-ReduceScatter (bo = b_O/4 per member);
            # just ship each bf16 chunk out. The host casts to f32.
            for j in range(4):
                rsb = op_.tile([P, D], dt.bfloat16, tag="rsb")
                nc.gpsimd.dma_start(rsb[:], rs_out.ap()[P * j:P * (j + 1), :])
                nc.gpsimd.dma_start(out_d.ap()[P * j:P * (j + 1), :], rsb[:])

    nc.compile()
    return nc


def _get_nc():
    if "nc" not in _NC_CACHE:
        _NC_CACHE["nc"] = build_kernel()
    return _NC_CACHE["nc"]


def make_in_maps(normalized_resid_pre, W_Q, W_K, W_V, W_O, b_Q, b_K, b_V, b_O):
    x = np.asarray(normalized_resid_pre, dtype=F32)
    W_Q = np.asarray(W_Q, F32); W_K = np.asarray(W_K, F32)
    W_V = np.asarray(W_V, F32); W_O = np.asarray(W_O, F32)
    b_Q = np.asarray(b_Q, F32); b_K = np.asarray(b_K, F32)
    b_V = np.asarray(b_V, F32); b_O = np.asarray(b_O, F32)

    sq = 1.0 / (D * np.sqrt(DH))            # folded into W_Q / b_Q
    so = 1.0 / (np.sqrt(D) * np.sqrt(DH))   # folded into W_O

    wo_s = (W_O * so).reshape(H, DH, D)
    tri = np.triu(np.ones((P, P), dtype=F32)).astype(BF16)  # tri[k,q]=1 iff k<=q
    bo_b = np.ascontiguousarray(np.broadcast_to(b_O / 4.0, (P, D))).astype(F32)

    in_maps = []
    for c in CORE_IDS:
        b = c // 4
        hg = c % 4
        hs = slice(HC * hg, HC * (hg + 1))
        xT_b = np.ascontiguousarray(x[b].T.astype(BF16))          # [D, S]
        wq_c = np.ascontiguousarray(
            (W_Q[hs] * sq).transpose(1, 0, 2).reshape(D, E).astype(BF16))
        wk_c = np.ascontiguousarray(
            W_K[hs].transpose(1, 0, 2).reshape(D, E).astype(BF16))
        wv_c = np.ascontiguousarray(
            W_V[hs].transpose(1, 0, 2).reshape(D, E).astype(BF16))
        wo_c = np.ascontiguousarray(
            wo_s[hs].reshape(E, D).astype(BF16))
        bq_c = np.ascontiguousarray(
            (b_Q[hs] * sq).reshape(E).reshape(2, P).T).astype(F32)  # [P, 2]
        bk_c = np.ascontiguousarray(
            b_K[hs].reshape(E).reshape(2, P).T).astype(F32)
        bv_c = np.ascontiguousarray(
            np.broadcast_to(b_V[hs].reshape(E), (P, E))).astype(F32)
        in_maps.append({
            "xT": xT_b, "wq": wq_c, "wk": wk_c, "wv": wv_c, "wo": wo_c,
            "bq": bq_c, "bk": bk_c, "bv": bv_c, "bo": bo_b, "tri": tri,
        })
    return in_maps


def assemble_out(results):
    out = np.empty((B, S, D), dtype=F32)
    for c in CORE_IDS:
        b, g = c // 4, c % 4
        r = results[c]["out"].astype(F32)  # bf16 chunks; j-th = q rows 512j+128g
        for j in range(4):
            out[b, QB * j + P * g:QB * j + P * (g + 1), :] = \
                r[P * j:P * (j + 1)]
    return out


def kernel(**inputs):
    from concourse.bass_utils import run_bass_kernel_spmd

    nc = _get_nc()
    in_maps = make_in_maps(**inputs)
    trace = bool(int(os.environ.get("BASS_KERNEL_TRACE", "0")))
    res = run_bass_kernel_spmd(nc, in_maps, CORE_IDS, trace=trace)
    _NC_CACHE["last_result"] = res
    return assemble_out(res.results)
